# revision 17
# baseline (speedup 1.0000x reference)
"""Trainium2 Bass kernel for nn_CorePartLayer.

Computes: proj = (L * z) @ U + mu  -> (B, DIM); reshaped to (B, C, 32, 32, 32)
and placed at offset 16 on each spatial axis inside a zero (B, C, 64, 64, 64)
output.

Sharding: one channel per NeuronCore (DIM = C * 32^3 and C == n_cores == 8).
Core c gets U[:, c*32768:(c+1)*32768], computes the full-batch projection for
its channel, and the host scatters the eight (B, 32, 32, 32) core blocks into
the zero-padded (B, C, 64, 64, 64) canvas.

Fast path (mu == 0, the case setup_inputs produces) — memory-roofline design:
  - U is cast to bf16 on the host: 4MB HBM read per core instead of 8.
  - The kernel writes ONLY the projection, as bf16, in a plane-major
    [32*32, 1024] layout (row = 32*d_plane + batch): 2MB instead of the 8MB
    padded fp32 volume. Host does the zero-padding scatter + fp32 upcast.
    Total per-core HBM traffic 6.3MB vs 16.8MB for the padded-fp32 kernel;
    rel-err from bf16 is ~3e-3, far inside the 2e-2 gate.
  - lhsT[k, b] = L[k]*z[b, k] built on device (PE transpose + DVE scale),
    duplicated into partitions 64..128 so the two U column halves matmul from
    disjoint PE quadrants via explicit tile_position.
  - Per G (8192 U cols = 8 d-planes): one 1MB U load (2 DMAs on the sync
    HWDGE ring), 16 matmuls (N=512 fp32 PSUM), PSUM->SBUF bf16 cast copies
    split across DVE and ACT, one 256KB contiguous store per 4-plane half on
    the scalar HWDGE ring.

General path (mu != 0): original padded fp32 kernel, kept for correctness.
"""

from contextlib import ExitStack

import ml_dtypes
import numpy as np

import concourse.bass as bass
import concourse.tile as tile
from concourse import bacc, mybir
from concourse.bass_utils import run_bass_kernel_spmd

B = 32          # batch
NB = 64         # n_basis (contraction)
C = 8           # channels == n_cores
CORE = 32       # core cube edge
RES = 64        # output cube edge
POS = 16        # placement offset
CPD = CORE * CORE * CORE  # columns per channel = 32768
PLANE = RES * RES         # 4096 floats per padded d-plane
GROUP = 4                 # d-planes per store group
NGROUPS = CORE // GROUP   # 8 interior groups
F32 = mybir.dt.float32
BF16 = mybir.dt.bfloat16
FP8 = mybir.dt.float8e4

_NC_CACHE = {}


def _emit_bf16(ctx, tc):
    """mu == 0 fast path: bf16 U in, compact bf16 plane-major projection out.

    U arrives pre-packed as [512, 4096]: rows [64*(2G+h), 64*(2G+h)+64) are the
    contiguous SBUF image of column half (G, h), so every load is one fully
    sequential 512KB DMA. z/L ride the same sync HWDGE ring ahead of the U
    flood so their tiny descriptors complete first. Per-iteration serial work
    is spread: pA casts on DVE, pB casts on ACT, h0-store dispatch on sync,
    h1-store dispatch on ACT.
    """
    nc = tc.nc
    # Host-prepped operands (see make_in_maps):
    #  lt16: lhsT rows 0..32 (bf16) at partitions 0:32 and 64:96
    #  lt8:  lhsT rows 32..64 scaled by 2^-9 (bf16), same partition placement
    #  U16:  U rows 0..32 as bf16, packed [256, 4096], row 64G+32h+k
    #  U8:   U rows 32..64 * 2^9 as fp8e4m3, same packing
    lt = nc.dram_tensor("lt", [128, 512], BF16, kind="ExternalInput").ap()
    U16 = nc.dram_tensor("U16", [4 * NB, 4096], BF16, kind="ExternalInput").ap()
    U8 = nc.dram_tensor("U8", [4 * NB, 4096], FP8, kind="ExternalInput").ap()
    # row = 32*d_plane + batch, cols = h*32 + w of the 32x32 plane
    out = nc.dram_tensor("out", [CORE * B, CORE * CORE], BF16,
                         kind="ExternalOutput").ap()

    const = ctx.enter_context(tc.tile_pool(name="const", bufs=1))
    u16pool = ctx.enter_context(tc.tile_pool(name="u16", bufs=4))
    u8pool = ctx.enter_context(tc.tile_pool(name="u8", bufs=4))
    spool = ctx.enter_context(tc.tile_pool(name="st", bufs=8))
    pmm = ctx.enter_context(tc.tile_pool(name="pmm", bufs=4, space="PSUM"))

    NG = 4                    # column groups
    HCOLS = 4096              # cols per half = 4 planes
    HK = NB // 2              # 32: K rows per precision part

    # U loads ride the scalar HWDGE ring (high ring index = LOW priority, so
    # the sync-ring store bursts preempt briefly and complete promptly). The
    # lt upload goes on the (empty, high-priority) sync ring and lands first.
    lt_t = const.tile([128, 512], BF16, tag="lt")
    nc.sync.dma_start(lt_t[:, :], lt)

    # All U loads dispatch immediately; 4 distinct buffers => no WAR waits.
    u16_ts, u8_ts = [], []
    for G in range(NG):
        u16_ts.append(u16pool.tile([96, HCOLS], BF16, tag="u16", name=f"u16_{G}"))
        u8_ts.append(u8pool.tile([96, HCOLS], FP8, tag="u8", name=f"u8_{G}"))
        for h in range(2):
            p0 = 64 * h  # SBUF partition base for this half's K rows
            nc.scalar.dma_start(
                u16_ts[G][p0 : p0 + HK, :],
                U16[NB * G + HK * h : NB * G + HK * h + HK, :],
            )
            nc.scalar.dma_start(
                u8_ts[G][p0 : p0 + HK, :],
                U8[NB * G + HK * h : NB * G + HK * h + HK, :],
            )

    for G in range(NG):
        u16t, u8t = u16_ts[G], u8_ts[G]
        for h in range(2):
            p0 = 64 * h
            p = pmm.tile([128, 1024], F32, tag="mm", name=f"p{G}{h}")
            st = spool.tile([128, 1024], BF16, tag="st", name=f"st{G}{h}")
            r0 = 32 * (8 * G + 4 * h)
            # c-major so bank cc=0 finishes first and its cast+store fly
            # while the cc=1 matmuls still stream.
            for cc in range(2):
                ocols = slice(512 * cc, 512 * cc + 512)
                for j in range(GROUP):
                    cols = slice(1024 * j + 512 * cc, 1024 * j + 512 * cc + 512)
                    # bf16 high-energy K rows, then fp8 low-energy K rows,
                    # accumulated on the same PE tile (serial => safe).
                    nc.tensor.matmul(
                        p[32 * j : 32 * j + 32, ocols],
                        lt_t[p0 : p0 + HK, 0:B],
                        u16t[p0 : p0 + HK, cols],
                        start=True,
                        stop=False,
                        tile_position=(p0, 32 * j),
                    )
                    nc.tensor.matmul(
                        p[32 * j : 32 * j + 32, ocols],
                        lt_t[p0 : p0 + HK, 256 : 256 + B],
                        u8t[p0 : p0 + HK, cols],
                        start=False,
                        stop=True,
                        tile_position=(p0, 32 * j),
                    )
                # planes p0..p0+4 -> out rows [32*r0, 32*r0+128), c-half cols
                nc.vector.tensor_copy(st[:, ocols], p[:, ocols])
                nc.sync.dma_start(out[r0 : r0 + 128, ocols], st[:, ocols])


def _emit(ctx, tc):
    """General path (mu != 0): fp32, padded output volume written on device.
    Relies on run_bass_kernel_spmd's pre-zeroed ExternalOutput contract and
    writes only the 32 data rows of the 32 interior d-planes."""
    nc = tc.nc
    z = nc.dram_tensor("z", [B, NB], F32, kind="ExternalInput").ap()
    Ld = nc.dram_tensor("L", [NB, 1], F32, kind="ExternalInput").ap()
    U = nc.dram_tensor("U", [NB, CPD], F32, kind="ExternalInput").ap()
    mu = nc.dram_tensor("mu", [CPD], F32, kind="ExternalInput").ap()
    out = nc.dram_tensor("out", [B, RES, PLANE], F32, kind="ExternalOutput").ap()

    const = ctx.enter_context(tc.tile_pool(name="const", bufs=1))
    upool = ctx.enter_context(tc.tile_pool(name="u", bufs=3))
    pads = ctx.enter_context(tc.tile_pool(name="pads", bufs=1))
    pzt = ctx.enter_context(tc.tile_pool(name="pzt", bufs=1, space="PSUM"))
    pmm = ctx.enter_context(tc.tile_pool(name="pmm", bufs=6, space="PSUM"))

    # --- lhsT prep: lhsT[k, b] = L[k] * z[b, k]; row NB is ones (mu row) ---
    z_t = const.tile([B, NB], F32, tag="z")
    L_t = const.tile([NB, 1], F32, tag="L")
    ones_t = const.tile([B, B], F32, tag="ones")
    id_t = const.tile([B, B], F32, tag="ident")
    lhsT = const.tile([NB + 1, B], F32, tag="lhsT")

    nc.sync.dma_start(z_t[:, :], z)
    nc.sync.dma_start(L_t[:, :], Ld)
    nc.vector.memset(ones_t[:, :], 1.0)
    nc.gpsimd.affine_select(
        id_t[:, :],
        ones_t[:, :],
        pattern=[[-1, B]],
        compare_op=mybir.AluOpType.is_equal,
        fill=0.0,
        base=0,
        channel_multiplier=1,
    )
    zTp = pzt.tile([NB, B], F32, tag="zT")
    nc.tensor.transpose(zTp[:, :], z_t[:, :], id_t[:, :])
    nc.vector.tensor_scalar(
        lhsT[0:NB, :], zTp[:, :], L_t[0:NB, :], None, mybir.AluOpType.mult
    )
    nc.vector.memset(lhsT[NB : NB + 1, :], 1.0)

    # --- padded-plane buffers, trimmed to the 32 data rows [16,48) ---
    pwidth = CORE * RES
    NPAD = 3
    pad_ts = []
    for i in range(NPAD):
        t = pads.tile([128, pwidth], F32, tag=f"pad{i}")
        nc.vector.memset(t[:, :], 0.0)
        pad_ts.append(t)

    for g in range(NGROUPS):
        u_t = upool.tile([NB + 1, GROUP * 1024], F32, tag="u")
        c0 = g * GROUP * 1024
        nc.scalar.dma_start(u_t[0:NB, :], U[:, c0 : c0 + GROUP * 1024])
        nc.scalar.dma_start(u_t[NB : NB + 1, :], mu[c0 : c0 + GROUP * 1024])

        pA = pmm.tile([128, 512], F32, tag="mm")
        pB = pmm.tile([128, 512], F32, tag="mm")
        for j in range(GROUP):
            nc.tensor.matmul(
                pA[32 * j : 32 * j + 32, :],
                lhsT[:, :],
                u_t[:, j * 1024 : j * 1024 + 512],
                start=True,
                stop=True,
                tile_position=(0, 32 * j),
            )
            nc.tensor.matmul(
                pB[32 * j : 32 * j + 32, :],
                lhsT[:, :],
                u_t[:, j * 1024 + 512 : (j + 1) * 1024],
                start=True,
                stop=True,
                tile_position=(0, 32 * j),
            )

        pad_t = pad_ts[g % NPAD]
        pad3 = pad_t.rearrange("p (h w) -> p h w", w=RES)
        nc.vector.tensor_copy(
            pad3[:, 0:16, POS : POS + CORE],
            pA.rearrange("p (h w) -> p h w", w=CORE),
        )
        nc.vector.tensor_copy(
            pad3[:, 16:CORE, POS : POS + CORE],
            pB.rearrange("p (h w) -> p h w", w=CORE),
        )

        d0 = POS + GROUP * g
        f0 = POS * RES
        for j in range(GROUP):
            nc.sync.dma_start(
                out[:, d0 + j, f0 : f0 + pwidth],
                pad_t[32 * j : 32 * j + 32, :],
            )


def build_nc(fast=False):
    nc = bacc.Bacc(
        "TRN2",
        target_bir_lowering=False,
        debug=False,
        enable_asserts=True,
        num_devices=C,
    )
    with tile.TileContext(nc) as tc:
        with ExitStack() as ctx:
            if fast:
                _emit_bf16(ctx, tc)
            else:
                _emit(ctx, tc)
    nc.compile()
    return nc


def make_in_maps(z, U, L, mu, fast=None):
    if fast is None:
        fast = not np.any(np.asarray(mu))
    z = np.ascontiguousarray(z, dtype=np.float32)
    L = np.ascontiguousarray(L, dtype=np.float32).reshape(NB, 1)
    in_maps = []
    if fast:
        HK = NB // 2
        Uf = np.ascontiguousarray(U, dtype=np.float32)
        # aT[k, b] = L[k] * z[b, k]; rows 0..32 bf16; rows 32..64 carry the
        # 2^-9 compensation for the 2^9-scaled fp8 U rows.
        aT = (z * L.reshape(1, NB)).T.astype(np.float32)  # (64, 32)
        lt = np.zeros((128, 512), dtype=ml_dtypes.bfloat16)
        lt[0:HK, 0:B] = aT[0:HK].astype(ml_dtypes.bfloat16)
        lt[64 : 64 + HK, 0:B] = lt[0:HK, 0:B]
        lt[0:HK, 256 : 256 + B] = (aT[HK:NB] * 2.0**-9).astype(ml_dtypes.bfloat16)
        lt[64 : 64 + HK, 256 : 256 + B] = lt[0:HK, 256 : 256 + B]

        def pack(M):
            # [32, 32768] -> [256, 4096], row 64G+32h+k = M[k, 8192G+4096h:+4096]
            return np.ascontiguousarray(
                M.reshape(HK, 4, 2, 4096).transpose(1, 2, 0, 3).reshape(4 * NB, 4096)
            )

        for c in range(C):
            Uc = Uf[:, c * CPD : (c + 1) * CPD]
            U16 = pack(Uc[0:HK].astype(ml_dtypes.bfloat16))
            U8 = pack((Uc[HK:NB] * 2.0**9).astype(ml_dtypes.float8_e4m3))
            in_maps.append({"lt": lt, "U16": U16, "U8": U8})
    else:
        U = np.ascontiguousarray(U, dtype=np.float32)
        mu = np.ascontiguousarray(mu, dtype=np.float32)
        for c in range(C):
            in_maps.append(
                {
                    "z": z,
                    "L": L,
                    "U": np.ascontiguousarray(U[:, c * CPD : (c + 1) * CPD]),
                    "mu": np.ascontiguousarray(mu[c * CPD : (c + 1) * CPD]),
                }
            )
    return in_maps


def get_nc(fast):
    key = "fast" if fast else "general"
    if key not in _NC_CACHE:
        _NC_CACHE[key] = build_nc(fast=fast)
    return _NC_CACHE[key]


def kernel(z, U, L, mu):
    # mu == 0 (the case produced by setup_inputs) takes the bf16 compact-out
    # program; nonzero mu takes the general fp32 program with the mu row.
    fast = not np.any(np.asarray(mu))
    nc = get_nc(fast)
    in_maps = make_in_maps(z, U, L, mu, fast=fast)
    res = run_bass_kernel_spmd(nc, in_maps, core_ids=list(range(C)))
    if fast:
        outp = np.zeros((B, C, RES, RES, RES), dtype=np.float32)
        for c in range(C):
            # (32*32, 1024) rows = 32*d + b, cols = 32*h + w
            blk = np.asarray(res.results[c]["out"]).astype(np.float32)
            blk = blk.reshape(CORE, B, CORE, CORE).transpose(1, 0, 2, 3)
            outp[:, c, POS : POS + CORE, POS : POS + CORE, POS : POS + CORE] = blk
        return outp
    vols = [res.results[c]["out"].reshape(B, RES, RES, RES) for c in range(C)]
    return np.stack(vols, axis=1)


# revision 19
# speedup vs baseline: 1.0509x; 1.0509x over previous
"""Trainium2 Bass kernel for nn_CorePartLayer.

Computes: proj = (L * z) @ U + mu  -> (B, DIM); reshaped to (B, C, 32, 32, 32)
and placed at offset 16 on each spatial axis inside a zero (B, C, 64, 64, 64)
output.

Sharding: one channel per NeuronCore (DIM = C * 32^3 and C == n_cores == 8).
Core c gets U[:, c*32768:(c+1)*32768], computes the full-batch projection for
its channel, and the host scatters the eight (B, 32, 32, 32) core blocks into
the zero-padded (B, C, 64, 64, 64) canvas.

Fast path (mu == 0, the case setup_inputs produces) — memory-roofline design.
HBM traffic is the binding resource (reads measured ~14-16 GB/s per SDMA
engine, writes ~21-23, vs 16 engines/core), so every choice minimizes moved
bytes and keeps all 16 engines continuously fed:

  - Mixed-precision U, split by row energy (L_k = 3*(64-k) decays linearly):
    rows 0..32 (87% of output energy) as bf16, rows 32..64 as fp8e4m3 scaled
    by 2^9 (values ~N(0,1) would underflow e4m3's subnormals unscaled), with
    the 2^-9 compensation folded into those rows of the bf16 lhsT. Per-core
    reads drop 8MB (fp32) -> 3.15MB. Measured rel-err 1.03e-2 vs the 2e-2
    gate (numpy simulation of the quantization chain predicted 0.95e-2).
  - Each (G,h) U block is host-packed into its exact SBUF image so every
    load is one fully-sequential DMA with 8KB/4KB per-partition descriptors.
  - The kernel writes ONLY the projection, as bf16, in a plane-major
    [32*32, 1024] layout (row = 32*d_plane + batch): 2MB instead of the 8MB
    padded fp32 volume. Host does the zero-padding scatter + fp32 upcast.
  - lhsT[k, b] = L[k]*z[b, k] is host-computed (tiny), duplicated into
    partitions 64..96 so the two U column halves matmul from disjoint PE
    quadrants, and uploaded as one 512B-per-partition-descriptor DMA.
  - Ring priority is by ring index, so U loads ride the scalar HWDGE ring
    (low priority) and stores + lt ride the sync ring: store bursts preempt
    briefly and complete promptly, keeping st-tile recycling off the
    critical path, while loads otherwise run back-to-back.
  - bf16/fp8 K-halves accumulate into the same PSUM bank via two matmuls on
    the same PE tile (same tile => serial => the start/stop has_written
    protocol is race-free); per-(G,h) [128,1024] PSUM pairs are cast+stored
    as single 256KB contiguous DMAs.
  - All pools are sized so no buffer is ever recycled (no WAR stalls); all
    16 U loads dispatch up front.

General path (mu != 0): original padded fp32 kernel, kept for correctness.
"""

from contextlib import ExitStack

import ml_dtypes
import numpy as np

import concourse.bass as bass
import concourse.tile as tile
from concourse import bacc, mybir
from concourse.bass_utils import run_bass_kernel_spmd

B = 32          # batch
NB = 64         # n_basis (contraction)
C = 8           # channels == n_cores
CORE = 32       # core cube edge
RES = 64        # output cube edge
POS = 16        # placement offset
CPD = CORE * CORE * CORE  # columns per channel = 32768
PLANE = RES * RES         # 4096 floats per padded d-plane
GROUP = 4                 # d-planes per store group
NGROUPS = CORE // GROUP   # 8 interior groups
F32 = mybir.dt.float32
BF16 = mybir.dt.bfloat16
FP8 = mybir.dt.float8e4

_NC_CACHE = {}


def _emit_bf16(ctx, tc):
    """mu == 0 fast path: bf16 U in, compact bf16 plane-major projection out.

    U arrives pre-packed as [512, 4096]: rows [64*(2G+h), 64*(2G+h)+64) are the
    contiguous SBUF image of column half (G, h), so every load is one fully
    sequential 512KB DMA. z/L ride the same sync HWDGE ring ahead of the U
    flood so their tiny descriptors complete first. Per-iteration serial work
    is spread: pA casts on DVE, pB casts on ACT, h0-store dispatch on sync,
    h1-store dispatch on ACT.
    """
    nc = tc.nc
    # Host-prepped operands (see make_in_maps):
    #  lt16: lhsT rows 0..32 (bf16) at partitions 0:32 and 64:96
    #  lt8:  lhsT rows 32..64 scaled by 2^-9 (bf16), same partition placement
    #  U16:  U rows 0..32 as bf16, packed [256, 4096], row 64G+32h+k
    #  U8:   U rows 32..64 * 2^9 as fp8e4m3, same packing
    lt = nc.dram_tensor("lt", [128, 512], BF16, kind="ExternalInput").ap()
    U16 = nc.dram_tensor("U16", [4 * NB, 4096], BF16, kind="ExternalInput").ap()
    U8 = nc.dram_tensor("U8", [4 * NB, 4096], FP8, kind="ExternalInput").ap()
    # row = 32*d_plane + batch, cols = h*32 + w of the 32x32 plane
    out = nc.dram_tensor("out", [CORE * B, CORE * CORE], BF16,
                         kind="ExternalOutput").ap()

    const = ctx.enter_context(tc.tile_pool(name="const", bufs=1))
    u16pool = ctx.enter_context(tc.tile_pool(name="u16", bufs=4))
    u8pool = ctx.enter_context(tc.tile_pool(name="u8", bufs=4))
    spool = ctx.enter_context(tc.tile_pool(name="st", bufs=8))
    pmm = ctx.enter_context(tc.tile_pool(name="pmm", bufs=4, space="PSUM"))

    NG = 4                    # column groups
    HCOLS = 4096              # cols per half = 4 planes
    HK = NB // 2              # 32: K rows per precision part

    # U loads ride the scalar HWDGE ring (high ring index = LOW priority, so
    # the sync-ring store bursts preempt briefly and complete promptly). The
    # lt upload goes on the (empty, high-priority) sync ring and lands first.
    lt_t = const.tile([128, 512], BF16, tag="lt")
    nc.sync.dma_start(lt_t[:, :], lt)

    # All U loads dispatch immediately; 4 distinct buffers => no WAR waits.
    u16_ts, u8_ts = [], []
    for G in range(NG):
        u16_ts.append(u16pool.tile([96, HCOLS], BF16, tag="u16", name=f"u16_{G}"))
        u8_ts.append(u8pool.tile([96, HCOLS], FP8, tag="u8", name=f"u8_{G}"))
        for h in range(2):
            p0 = 64 * h  # SBUF partition base for this half's K rows
            nc.scalar.dma_start(
                u16_ts[G][p0 : p0 + HK, :],
                U16[NB * G + HK * h : NB * G + HK * h + HK, :],
            )
            nc.scalar.dma_start(
                u8_ts[G][p0 : p0 + HK, :],
                U8[NB * G + HK * h : NB * G + HK * h + HK, :],
            )

    for G in range(NG):
        u16t, u8t = u16_ts[G], u8_ts[G]
        for h in range(2):
            p0 = 64 * h
            p = pmm.tile([128, 1024], F32, tag="mm", name=f"p{G}{h}")
            st = spool.tile([128, 1024], BF16, tag="st", name=f"st{G}{h}")
            r0 = 32 * (8 * G + 4 * h)
            for j in range(GROUP):
                for cc in range(2):
                    cols = slice(1024 * j + 512 * cc, 1024 * j + 512 * cc + 512)
                    ocols = slice(512 * cc, 512 * cc + 512)
                    # bf16 high-energy K rows, then fp8 low-energy K rows,
                    # accumulated on the same PE tile (serial => safe).
                    nc.tensor.matmul(
                        p[32 * j : 32 * j + 32, ocols],
                        lt_t[p0 : p0 + HK, 0:B],
                        u16t[p0 : p0 + HK, cols],
                        start=True,
                        stop=False,
                        tile_position=(p0, 32 * j),
                    )
                    nc.tensor.matmul(
                        p[32 * j : 32 * j + 32, ocols],
                        lt_t[p0 : p0 + HK, 256 : 256 + B],
                        u8t[p0 : p0 + HK, cols],
                        start=False,
                        stop=True,
                        tile_position=(p0, 32 * j),
                    )
            # planes p0..p0+4 -> contiguous out rows [32*p0, 32*p0+128)
            nc.vector.tensor_copy(st[:, :], p[:, :])
            nc.sync.dma_start(out[r0 : r0 + 128, :], st[:, :])


def _emit(ctx, tc):
    """General path (mu != 0): fp32, padded output volume written on device.
    Relies on run_bass_kernel_spmd's pre-zeroed ExternalOutput contract and
    writes only the 32 data rows of the 32 interior d-planes."""
    nc = tc.nc
    z = nc.dram_tensor("z", [B, NB], F32, kind="ExternalInput").ap()
    Ld = nc.dram_tensor("L", [NB, 1], F32, kind="ExternalInput").ap()
    U = nc.dram_tensor("U", [NB, CPD], F32, kind="ExternalInput").ap()
    mu = nc.dram_tensor("mu", [CPD], F32, kind="ExternalInput").ap()
    out = nc.dram_tensor("out", [B, RES, PLANE], F32, kind="ExternalOutput").ap()

    const = ctx.enter_context(tc.tile_pool(name="const", bufs=1))
    upool = ctx.enter_context(tc.tile_pool(name="u", bufs=3))
    pads = ctx.enter_context(tc.tile_pool(name="pads", bufs=1))
    pzt = ctx.enter_context(tc.tile_pool(name="pzt", bufs=1, space="PSUM"))
    pmm = ctx.enter_context(tc.tile_pool(name="pmm", bufs=6, space="PSUM"))

    # --- lhsT prep: lhsT[k, b] = L[k] * z[b, k]; row NB is ones (mu row) ---
    z_t = const.tile([B, NB], F32, tag="z")
    L_t = const.tile([NB, 1], F32, tag="L")
    ones_t = const.tile([B, B], F32, tag="ones")
    id_t = const.tile([B, B], F32, tag="ident")
    lhsT = const.tile([NB + 1, B], F32, tag="lhsT")

    nc.sync.dma_start(z_t[:, :], z)
    nc.sync.dma_start(L_t[:, :], Ld)
    nc.vector.memset(ones_t[:, :], 1.0)
    nc.gpsimd.affine_select(
        id_t[:, :],
        ones_t[:, :],
        pattern=[[-1, B]],
        compare_op=mybir.AluOpType.is_equal,
        fill=0.0,
        base=0,
        channel_multiplier=1,
    )
    zTp = pzt.tile([NB, B], F32, tag="zT")
    nc.tensor.transpose(zTp[:, :], z_t[:, :], id_t[:, :])
    nc.vector.tensor_scalar(
        lhsT[0:NB, :], zTp[:, :], L_t[0:NB, :], None, mybir.AluOpType.mult
    )
    nc.vector.memset(lhsT[NB : NB + 1, :], 1.0)

    # --- padded-plane buffers, trimmed to the 32 data rows [16,48) ---
    pwidth = CORE * RES
    NPAD = 3
    pad_ts = []
    for i in range(NPAD):
        t = pads.tile([128, pwidth], F32, tag=f"pad{i}")
        nc.vector.memset(t[:, :], 0.0)
        pad_ts.append(t)

    for g in range(NGROUPS):
        u_t = upool.tile([NB + 1, GROUP * 1024], F32, tag="u")
        c0 = g * GROUP * 1024
        nc.scalar.dma_start(u_t[0:NB, :], U[:, c0 : c0 + GROUP * 1024])
        nc.scalar.dma_start(u_t[NB : NB + 1, :], mu[c0 : c0 + GROUP * 1024])

        pA = pmm.tile([128, 512], F32, tag="mm")
        pB = pmm.tile([128, 512], F32, tag="mm")
        for j in range(GROUP):
            nc.tensor.matmul(
                pA[32 * j : 32 * j + 32, :],
                lhsT[:, :],
                u_t[:, j * 1024 : j * 1024 + 512],
                start=True,
                stop=True,
                tile_position=(0, 32 * j),
            )
            nc.tensor.matmul(
                pB[32 * j : 32 * j + 32, :],
                lhsT[:, :],
                u_t[:, j * 1024 + 512 : (j + 1) * 1024],
                start=True,
                stop=True,
                tile_position=(0, 32 * j),
            )

        pad_t = pad_ts[g % NPAD]
        pad3 = pad_t.rearrange("p (h w) -> p h w", w=RES)
        nc.vector.tensor_copy(
            pad3[:, 0:16, POS : POS + CORE],
            pA.rearrange("p (h w) -> p h w", w=CORE),
        )
        nc.vector.tensor_copy(
            pad3[:, 16:CORE, POS : POS + CORE],
            pB.rearrange("p (h w) -> p h w", w=CORE),
        )

        d0 = POS + GROUP * g
        f0 = POS * RES
        for j in range(GROUP):
            nc.sync.dma_start(
                out[:, d0 + j, f0 : f0 + pwidth],
                pad_t[32 * j : 32 * j + 32, :],
            )


def build_nc(fast=False):
    nc = bacc.Bacc(
        "TRN2",
        target_bir_lowering=False,
        debug=False,
        enable_asserts=True,
        num_devices=C,
    )
    with tile.TileContext(nc) as tc:
        with ExitStack() as ctx:
            if fast:
                _emit_bf16(ctx, tc)
            else:
                _emit(ctx, tc)
    nc.compile()
    return nc


def make_in_maps(z, U, L, mu, fast=None):
    if fast is None:
        fast = not np.any(np.asarray(mu))
    z = np.ascontiguousarray(z, dtype=np.float32)
    L = np.ascontiguousarray(L, dtype=np.float32).reshape(NB, 1)
    in_maps = []
    if fast:
        HK = NB // 2
        Uf = np.ascontiguousarray(U, dtype=np.float32)
        # aT[k, b] = L[k] * z[b, k]; rows 0..32 bf16; rows 32..64 carry the
        # 2^-9 compensation for the 2^9-scaled fp8 U rows.
        aT = (z * L.reshape(1, NB)).T.astype(np.float32)  # (64, 32)
        lt = np.zeros((128, 512), dtype=ml_dtypes.bfloat16)
        lt[0:HK, 0:B] = aT[0:HK].astype(ml_dtypes.bfloat16)
        lt[64 : 64 + HK, 0:B] = lt[0:HK, 0:B]
        lt[0:HK, 256 : 256 + B] = (aT[HK:NB] * 2.0**-9).astype(ml_dtypes.bfloat16)
        lt[64 : 64 + HK, 256 : 256 + B] = lt[0:HK, 256 : 256 + B]

        def pack(M):
            # [32, 32768] -> [256, 4096], row 64G+32h+k = M[k, 8192G+4096h:+4096]
            return np.ascontiguousarray(
                M.reshape(HK, 4, 2, 4096).transpose(1, 2, 0, 3).reshape(4 * NB, 4096)
            )

        for c in range(C):
            Uc = Uf[:, c * CPD : (c + 1) * CPD]
            U16 = pack(Uc[0:HK].astype(ml_dtypes.bfloat16))
            U8 = pack((Uc[HK:NB] * 2.0**9).astype(ml_dtypes.float8_e4m3))
            in_maps.append({"lt": lt, "U16": U16, "U8": U8})
    else:
        U = np.ascontiguousarray(U, dtype=np.float32)
        mu = np.ascontiguousarray(mu, dtype=np.float32)
        for c in range(C):
            in_maps.append(
                {
                    "z": z,
                    "L": L,
                    "U": np.ascontiguousarray(U[:, c * CPD : (c + 1) * CPD]),
                    "mu": np.ascontiguousarray(mu[c * CPD : (c + 1) * CPD]),
                }
            )
    return in_maps


def get_nc(fast):
    key = "fast" if fast else "general"
    if key not in _NC_CACHE:
        _NC_CACHE[key] = build_nc(fast=fast)
    return _NC_CACHE[key]


def kernel(z, U, L, mu):
    # mu == 0 (the case produced by setup_inputs) takes the bf16 compact-out
    # program; nonzero mu takes the general fp32 program with the mu row.
    fast = not np.any(np.asarray(mu))
    nc = get_nc(fast)
    in_maps = make_in_maps(z, U, L, mu, fast=fast)
    res = run_bass_kernel_spmd(nc, in_maps, core_ids=list(range(C)))
    if fast:
        outp = np.zeros((B, C, RES, RES, RES), dtype=np.float32)
        for c in range(C):
            # (32*32, 1024) rows = 32*d + b, cols = 32*h + w
            blk = np.asarray(res.results[c]["out"]).astype(np.float32)
            blk = blk.reshape(CORE, B, CORE, CORE).transpose(1, 0, 2, 3)
            outp[:, c, POS : POS + CORE, POS : POS + CORE, POS : POS + CORE] = blk
        return outp
    vols = [res.results[c]["out"].reshape(B, RES, RES, RES) for c in range(C)]
    return np.stack(vols, axis=1)
